# revision 7
# baseline (speedup 1.0000x reference)
"""Trainium2 Bass kernel for a 2-layer GCN encoder + MLP head (PyG GCNConv).

v2 strategy (8 NeuronCores, node-parallel):
  - Nodes sharded by contiguous range; dst tiles are contiguous 128-node
    blocks (T = ceil(SH/128)), so aggregation output writes are plain
    contiguous DMA (no indirect scatter).
  - norm = dinv[src]*dinv[dst] is factored: tables store dinv[i]*z[i]
    (scaled at conv evacuation); aggregation post-multiplies by dinv[d]
    fused with relu on DVE; bias enters via a 1-row matmul (sqdeg x b).
  - Self-loop edges form chunk 0 of each tile, loaded contiguously from the
    core-local table shard (z0in/z1in) with a plain dma_start.
  - Remaining edges: per tile two dma_gathers (lo/hi int16 index halves)
    with per-tile real counts (16-padded) and round-robin SWDGE queues
    (4 queues) to avoid descriptor-ring serialization.
  - One-hot routing matrices built on DVE with a single broadcast-AP
    is_equal per tile; aggregation = per-chunk one-hot matmuls in PSUM.
  - conv2 fused in the layer-1 tile loop via PE transposes.
All heavy compute bf16 with fp32 PSUM accumulation.
"""
import sys

for _p in ("/opt/trn_rl_repo",):
    if _p not in sys.path:
        sys.path.insert(0, _p)

import numpy as np
import ml_dtypes

bf16 = ml_dtypes.bfloat16

P = 128
H = 256          # gcn hidden width (fixed)
HH = 128         # head hidden width (fixed)
OH = 40          # wt_onehot + mut_onehot width (fixed)
NCORES = 8
NQ = 4           # SWDGE queues


class Cfg:
    def __init__(self, N, E, D_IN, B):
        self.N, self.E, self.D_IN, self.B = N, E, D_IN, B
        assert N % NCORES == 0
        self.SH = N // NCORES                      # real rows per shard
        self.T = -(-self.SH // P)                  # dst tiles per shard
        shp = max(self.T * P, -(-self.SH // P) * P)
        if shp == self.SH:
            shp += P
        self.SHP = -(-shp // P) * P                # padded rows per shard
        self.NP = NCORES * self.SHP                # padded global rows
        self.HSH = ((self.SHP // P + 1) // 2) * P  # A-half rows (P-aligned)
        self.HSB = self.SHP - self.HSH             # B-half rows
        self.TBL = NCORES * self.HSH               # A table rows
        self.TBB = NCORES * self.HSB               # B table rows
        assert self.TBL < 32768 and self.TBB < 32768
        self.KT = -(-D_IN // P)                    # k tiles for conv1
        self.KPAD = self.KT * P
        self.MT = self.SHP // P                    # m tiles per shard
        self.BPC = self.B // NCORES                # batch per core
        assert self.BPC % P == 0
        self.BCH = self.BPC // P                   # batch chunks


REAL = Cfg(N=50000, E=800000, D_IN=1281, B=4096)


# ---------------------------------------------------------------- host prep

def _pack_idx16(seq):
    """idx sequence [n] (n%16==0) -> wrapped-16 replicated [128, n//16] i16."""
    n = seq.shape[0]
    assert n % 16 == 0
    a = seq.reshape(n // 16, 16).T.astype(np.int16)
    return np.tile(a, (8, 1))


def host_prep(cfg, x, wt_onehot, mut_onehot, Wc1, bc1, Wc2, bc2,
              Wh1, bh1, Wh2, bh2, Wh3, bh3, edge_index, var_node_idx):
    N, E, SH, SHP, T = cfg.N, cfg.E, cfg.SH, cfg.SHP, cfg.T
    src = np.asarray(edge_index[0], np.int64)
    dst = np.asarray(edge_index[1], np.int64)
    # degree includes self loops (dst counts + 1)
    deg = (np.bincount(dst, minlength=N) + 1).astype(np.float32)
    dinv = (1.0 / np.sqrt(deg)).astype(np.float32)
    sqdeg = np.sqrt(deg).astype(np.float32)
    q_of = src // SH
    r_of = src % SH
    in_b = r_of >= cfg.HSH
    srcp = np.where(in_b, q_of * cfg.HSB + (r_of - cfg.HSH),
                    q_of * cfg.HSH + r_of)

    # drop self-referencing edges? (none in random data, but (i,i) entries in
    # edge_index are real edges distinct from the implicit self loop)
    core_of = dst // SH

    # per-core, per-tile edge lists
    per_core = []
    CLmax = CHmax = 0
    for q in range(NCORES):
        m = core_of == q
        d_loc = dst[m] - q * SH
        sp = srcp[m]
        tile_of = d_loc // P
        order = np.argsort(tile_of, kind="stable")
        d_s, sp_s, t_s = d_loc[order], sp[order], tile_of[order]
        tstarts = np.searchsorted(t_s, np.arange(T + 1))
        tiles = []
        inb_s = in_b[m][order]
        for t in range(T):
            a, b = tstarts[t], tstarts[t + 1]
            spt, dt_, ib = sp_s[a:b], d_s[a:b] - t * P, inb_s[a:b]
            lo = ~ib
            tiles.append(((spt[lo], dt_[lo]), (spt[ib], dt_[ib])))
            CLmax = max(CLmax, -(-int(lo.sum()) // P))
            CHmax = max(CHmax, -(-int(ib.sum()) // P))
        per_core.append(tiles)
    CL, CH = max(1, int(CLmax)), max(1, int(CHmax))
    C = 1 + CL + CH   # self chunk + lo chunks + hi chunks

    # per-tile static counts must be IDENTICAL across cores (one program):
    # use the max over cores for each tile's lo/hi counts.
    nlo = np.zeros(T, np.int64)
    nhi = np.zeros(T, np.int64)
    for q in range(NCORES):
        for t in range(T):
            (sl, _), (sh_, _) = per_core[q][t]
            nlo[t] = max(nlo[t], len(sl))
            nhi[t] = max(nhi[t], len(sh_))
    nlo16 = ((nlo + 15) // 16) * 16
    nhi16 = ((nhi + 15) // 16) * 16
    clo = np.maximum(1, -(-nlo16 // P))  # chunks per tile (>=1 for layout)
    chi = np.maximum(1, -(-nhi16 // P))
    # column offsets into gidx (units of 16-idx columns)
    lo_off = np.zeros(T + 1, np.int64)
    hi_off = np.zeros(T + 1, np.int64)
    np.cumsum(nlo16 // 16, out=lo_off[1:])
    hi_base = lo_off[T]
    np.cumsum(nhi16 // 16, out=hi_off[1:])
    gcols = int(lo_off[T] + hi_off[T])

    vni = np.asarray(var_node_idx, np.int64)
    vq, vr = vni // SH, vni % SH
    positions = [np.nonzero(vq == q)[0] for q in range(NCORES)]
    bmax = max(len(p) for p in positions)
    BMAX = ((bmax + P - 1) // P) * P

    meta = dict(CL=CL, CH=CH, C=C, BMAX=BMAX, positions=positions,
                nlo16=tuple(int(v) for v in nlo16),
                nhi16=tuple(int(v) for v in nhi16),
                clo=tuple(int(v) for v in clo),
                chi=tuple(int(v) for v in chi),
                gcols=gcols, hi_base=int(hi_base))

    # shared weights
    wc1 = np.zeros((cfg.KPAD, H), bf16)
    wc1[:cfg.D_IN] = np.asarray(Wc1, np.float32).astype(bf16)
    wc2 = np.asarray(Wc2, np.float32).astype(bf16)
    wh1 = np.zeros((3 * P, HH), bf16)
    wh1[:H + OH] = np.asarray(Wh1, np.float32).astype(bf16)
    wh2 = np.asarray(Wh2, np.float32).astype(bf16)
    wh3 = np.asarray(Wh3, np.float32).astype(bf16)
    b1row = np.asarray(bc1, np.float32).astype(bf16).reshape(1, H)
    b2row = np.asarray(bc2, np.float32).astype(bf16).reshape(1, H)
    bh1v = np.asarray(bh1, np.float32).reshape(HH, 1)
    bh2v = np.asarray(bh2, np.float32).reshape(HH // 2, 1)
    bh3v = np.asarray(bh3, np.float32).reshape(1, 1)

    x = np.asarray(x, np.float32)
    wt_b = np.asarray(wt_onehot, np.float32).astype(bf16)
    mut_b = np.asarray(mut_onehot, np.float32).astype(bf16)

    in_maps = []
    for q in range(NCORES):
        gidx_seq = np.zeros(gcols * 16, np.int64)
        dsel = np.full((P, T * C), 999.0, np.float32)
        for t in range(T):
            (sl, dl), (sh_, dh_) = per_core[q][t]
            # self chunk col: diagonal for real rows
            nreal = min(P, SH - t * P)
            dsel[:nreal, t * C] = np.arange(nreal, dtype=np.float32)
            base = lo_off[t] * 16
            gidx_seq[base:base + len(sl)] = sl
            a_d = np.full(clo[t] * P, 999.0, np.float32)
            a_d[:len(dl)] = dl
            dsel[:, t * C + 1:t * C + 1 + clo[t]] = \
                a_d.reshape(clo[t], P).T
            base = (hi_base + hi_off[t]) * 16
            gidx_seq[base:base + len(sh_)] = sh_
            a_d = np.full(chi[t] * P, 999.0, np.float32)
            a_d[:len(dh_)] = dh_
            dsel[:, t * C + 1 + clo[t]:t * C + 1 + clo[t] + chi[t]] = \
                a_d.reshape(chi[t], P).T
        gidx = _pack_idx16(gidx_seq)

        # dinv per m-tile column [128, MT]; sqdeg row [1, T*P]
        dloc = np.zeros(SHP, np.float32)
        dloc[:SH] = dinv[q * SH:(q + 1) * SH]
        dinv_tbl = dloc.reshape(cfg.MT, P).T.copy()
        sq = np.zeros(T * P, np.float32)
        sq[:SH] = sqdeg[q * SH:(q + 1) * SH]
        sqrow = sq.reshape(1, T * P).astype(bf16)

        xT = np.zeros((cfg.KPAD, SHP), bf16)
        xT[:cfg.D_IN, :SH] = x[q * SH:(q + 1) * SH].T.astype(bf16)
        pos = positions[q]
        vloc = np.zeros(BMAX, np.int64)
        vloc[:len(pos)] = vr[pos]
        vidx = vloc.reshape(BMAX // P, P).T.astype(np.int32)
        ohp = np.zeros((BMAX, 2 * 20), bf16)
        ohp[:len(pos), :20] = wt_b[pos]
        ohp[:len(pos), 20:] = mut_b[pos]
        ohT = ohp.T.copy()  # [40, BMAX]
        in_maps.append(dict(
            xT=xT, gidx=gidx, dsel=dsel.astype(bf16),
            dinv_tbl=np.ascontiguousarray(dinv_tbl),
            sqrow=np.ascontiguousarray(sqrow),
            vidx=np.ascontiguousarray(vidx), ohT=np.ascontiguousarray(ohT),
            wc1=wc1, wc2=wc2, wh1=wh1, wh2=wh2, wh3=wh3,
            b1row=b1row, b2row=b2row,
            bh1v=bh1v, bh2v=bh2v, bh3v=bh3v,
        ))
    return in_maps, meta


# ------------------------------------------------------------- bass program

def build_program(cfg, meta):
    import concourse.bass as bass
    import concourse.mybir as mybir
    import concourse.tile as tile
    from concourse import bacc
    from concourse.masks import make_identity

    T = cfg.T
    CL, CH, C = meta["CL"], meta["CH"], meta["C"]
    nlo16, nhi16 = meta["nlo16"], meta["nhi16"]
    clo, chi = meta["clo"], meta["chi"]
    gcols, hi_base = meta["gcols"], meta["hi_base"]
    BMAX = meta["BMAX"]
    BCH2 = BMAX // P

    nc = bacc.Bacc("TRN2", target_bir_lowering=False, debug=False,
                   num_devices=NCORES, num_swdge_queues=NQ,
                   dynamic_dma_scratch_size=49152)
    f32, bfl, i16, i32 = (mybir.dt.float32, mybir.dt.bfloat16,
                          mybir.dt.int16, mybir.dt.int32)
    fp8 = mybir.dt.float8e4

    # I/O
    xT = nc.dram_tensor("xT", [cfg.KPAD, cfg.SHP], bfl, kind="ExternalInput")
    gidx = nc.dram_tensor("gidx", [P, gcols], i16, kind="ExternalInput")
    dsel = nc.dram_tensor("dsel", [P, T * C], bfl, kind="ExternalInput")
    dinv_tbl = nc.dram_tensor("dinv_tbl", [P, cfg.MT], f32,
                              kind="ExternalInput")
    sqrow = nc.dram_tensor("sqrow", [1, T * P], bfl, kind="ExternalInput")
    vidx = nc.dram_tensor("vidx", [P, BCH2], i32, kind="ExternalInput")
    ohT = nc.dram_tensor("ohT", [OH, BMAX], bfl, kind="ExternalInput")
    wc1 = nc.dram_tensor("wc1", [cfg.KPAD, H], bfl, kind="ExternalInput")
    wc2 = nc.dram_tensor("wc2", [H, H], bfl, kind="ExternalInput")
    wh1 = nc.dram_tensor("wh1", [3 * P, HH], bfl, kind="ExternalInput")
    wh2 = nc.dram_tensor("wh2", [HH, HH // 2], bfl, kind="ExternalInput")
    wh3 = nc.dram_tensor("wh3", [HH // 2, 1], bfl, kind="ExternalInput")
    b1row = nc.dram_tensor("b1row", [1, H], bfl, kind="ExternalInput")
    b2row = nc.dram_tensor("b2row", [1, H], bfl, kind="ExternalInput")
    bh1v = nc.dram_tensor("bh1v", [HH, 1], f32, kind="ExternalInput")
    bh2v = nc.dram_tensor("bh2v", [HH // 2, 1], f32, kind="ExternalInput")
    bh3v = nc.dram_tensor("bh3v", [1, 1], f32, kind="ExternalInput")
    out = nc.dram_tensor("out", [1, BMAX], f32, kind="ExternalOutput")

    # internal DRAM
    z0in = nc.dram_tensor("z0in", [cfg.SHP, H], fp8, kind="Internal")
    z1in = nc.dram_tensor("z1in", [cfg.SHP, H], fp8, kind="Internal")
    h2in = nc.dram_tensor("h2in", [cfg.SHP, H], bfl, kind="Internal")
    Z0a = nc.dram_tensor("Z0a", [cfg.TBL, H], fp8, kind="Internal",
                         addr_space="Shared")
    Z0b = nc.dram_tensor("Z0b", [cfg.TBB, H], fp8, kind="Internal",
                         addr_space="Shared")
    Z1a = nc.dram_tensor("Z1a", [cfg.TBL, H], fp8, kind="Internal",
                         addr_space="Shared")
    Z1b = nc.dram_tensor("Z1b", [cfg.TBB, H], fp8, kind="Internal",
                         addr_space="Shared")
    rg = [list(range(NCORES))]

    with tile.TileContext(nc) as tc:
        with tc.tile_pool(name="const", bufs=1) as const:
            iota_i = const.tile([P, P], i32)
            nc.gpsimd.iota(iota_i[:], pattern=[[1, P]], base=0,
                           channel_multiplier=0)
            iota_b = const.tile([P, P], bfl)
            nc.vector.tensor_copy(iota_b[:], iota_i[:])
            ident = const.tile([P, P], bfl)
            make_identity(nc, ident[:])

            def load(ap, shape, dt):
                t = const.tile(shape, dt, tag=ap.tensor.name)
                nc.sync.dma_start(t[:], ap)
                return t

            wc1_sb = load(wc1.rearrange("(t p) n -> p t n", p=P)[:],
                          [P, cfg.KT, H], bfl)
            wc2_sb = load(wc2.rearrange("(t p) n -> p t n", p=P)[:],
                          [P, 2, H], bfl)
            wh1_sb = load(wh1.rearrange("(t p) n -> p t n", p=P)[:],
                          [P, 3, HH], bfl)
            wh2_sb = load(wh2[:], [HH, HH // 2], bfl)
            wh3_sb = load(wh3[:], [HH // 2, 1], bfl)
            b1_sb = load(b1row[:], [1, H], bfl)
            b2_sb = load(b2row[:], [1, H], bfl)
            bh1_sb = load(bh1v[:], [HH, 1], f32)
            bh2_sb = load(bh2v[:], [HH // 2, 1], f32)
            bh3_sb = load(bh3v[:], [1, 1], f32)
            gidx_sb = load(gidx[:], [P, gcols], i16)
            dsel_sb = load(dsel[:], [P, T * C], bfl)
            dinv_sb = load(dinv_tbl[:], [P, cfg.MT], f32)
            sqrow_sb = load(sqrow[:], [1, T * P], bfl)
            vidx_sb = load(vidx[:], [P, BCH2], i32)
            ohT_sb = load(ohT[:], [OH, BMAX], bfl)

            npad = cfg.SHP - cfg.T * P
            if npad > 0:
                zpad = const.tile([P, H], bfl)
                nc.any.memset(zpad[:], 0.0)
                zpad8 = const.tile([P, H], fp8)
                nc.any.memset(zpad8[:], 0.0)
                nc.sync.dma_start(z1in[cfg.T * P:cfg.SHP, :], zpad8[:npad, :])
                nc.sync.dma_start(h2in[cfg.T * P:cfg.SHP, :], zpad[:npad, :])

            # ---------------- phase A: conv1 z0 = dinv * (x @ Wc1)
            MBS = 7
            with tc.tile_pool(name="c1sb", bufs=3) as c1sb, \
                 tc.tile_pool(name="c1ev", bufs=3) as c1ev, \
                 tc.tile_pool(name="c1ps", bufs=MBS + 1, space="PSUM") as c1ps:
                for mb0 in range(0, cfg.MT, MBS):
                    mbn = min(MBS, cfg.MT - mb0)
                    accs = [c1ps.tile([P, H], f32, tag="convacc",
                                      name=f"convacc_{mb0}_{j}")
                            for j in range(mbn)]
                    for kt in range(cfg.KT):
                        slab = c1sb.tile([P, MBS * P], bfl, tag="slab")
                        nc.sync.dma_start(
                            slab[:, :mbn * P],
                            xT[kt * P:(kt + 1) * P, mb0 * P:(mb0 + mbn) * P])
                        for j in range(mbn):
                            nc.tensor.matmul(
                                accs[j][:], lhsT=slab[:, j * P:(j + 1) * P],
                                rhs=wc1_sb[:, kt, :],
                                start=(kt == 0), stop=(kt == cfg.KT - 1))
                    for j in range(mbn):
                        zb = c1ev.tile([P, H], fp8, tag="zev")
                        col = mb0 + j
                        nc.vector.tensor_scalar(
                            out=zb[:], in0=accs[j][:],
                            scalar1=dinv_sb[:, col:col + 1], scalar2=None,
                            op0=mybir.AluOpType.mult)
                        r0 = col * P
                        nc.sync.dma_start(z0in[r0:r0 + P, :], zb[:])
                    if mb0 + mbn >= cfg.HSH // P and mb0 < cfg.HSH // P:
                        nc.gpsimd.collective_compute(
                            "AllGather", mybir.AluOpType.bypass,
                            replica_groups=rg,
                            ins=[z0in[:cfg.HSH, :]], outs=[Z0a[:]])

            nc.gpsimd.collective_compute(
                "AllGather", mybir.AluOpType.bypass, replica_groups=rg,
                ins=[z0in[cfg.HSH:, :]], outs=[Z0b[:]])

            # ---------------- aggregation layers
            def agg_layer(Za, Zb, locin, b_sb, out_dram, do_conv2, lname,
                          mdt, odt, post_tile=None):
                with tc.tile_pool(name=f"agsb{lname}", bufs=10) as agsb, \
                     tc.tile_pool(name=f"agst{lname}", bufs=8) as agst, \
                     tc.tile_pool(name=f"agev{lname}", bufs=3) as agev, \
                     tc.tile_pool(name=f"agps{lname}", bufs=3,
                                  space="PSUM") as agps, \
                     tc.tile_pool(name=f"agp2{lname}", bufs=2,
                                  space="PSUM") as agp2:
                    qn = 0
                    for t in range(T):
                        ct = 1 + clo[t] + chi[t]
                        msg = agsb.tile([P, C, H], mdt, tag="msg")
                        # self chunk: contiguous local table rows
                        nc.sync.dma_start(msg[:, 0, :],
                                          locin[t * P:(t + 1) * P, :])
                        if nlo16[t] % P != 0 or nlo16[t] == 0:
                            nc.vector.memset(msg[:, 1 + nlo16[t] // P, :], 0.0)
                        if nlo16[t] > 0:
                            nc.gpsimd.dma_gather(
                                msg[:, 1:1 + clo[t], :], Za[:],
                                gidx_sb[:, lo_off_c[t]:lo_off_c[t + 1]],
                                nlo16[t], nlo16[t], H, single_packet=False,
                                queue_num=qn % NQ)
                            qn += 1
                        if nhi16[t] % P != 0 or nhi16[t] == 0:
                            nc.vector.memset(
                                msg[:, 1 + clo[t] + nhi16[t] // P, :], 0.0)
                        if nhi16[t] > 0:
                            nc.gpsimd.dma_gather(
                                msg[:, 1 + clo[t]:1 + clo[t] + chi[t], :],
                                Zb[:],
                                gidx_sb[:, hi_base + hi_off_c[t]:
                                        hi_base + hi_off_c[t + 1]],
                                nhi16[t], nhi16[t], H, single_packet=False,
                                queue_num=qn % NQ)
                            qn += 1
                        # one-hot build: st[p, c, j] = (dsel[p, c] == j)
                        st = agst.tile([P, C * P], mdt, tag="st")
                        dse = dsel_sb[:, t * C:t * C + ct]
                        dse_b = bass.AP(dse.tensor, dse.offset,
                                        [dse.ap[0], dse.ap[1], [0, P]])
                        io = iota_b[:]
                        io_b = bass.AP(io.tensor, io.offset,
                                       [io.ap[0], [0, ct], io.ap[1]])
                        ob = st[:, :ct * P].rearrange("p (c j) -> p c j", j=P)
                        nc.vector.tensor_tensor(out=ob, in0=io_b, in1=dse_b,
                                                op=mybir.AluOpType.is_equal)
                        acc = agps.tile([P, H], f32, tag="agacc")
                        nc.tensor.matmul(acc[:],
                                         lhsT=sqrow_sb[:, t * P:(t + 1) * P],
                                         rhs=b_sb[:], start=True, stop=False)
                        for ci in range(ct):
                            nc.tensor.matmul(acc[:],
                                             lhsT=st[:, ci * P:(ci + 1) * P],
                                             rhs=msg[:, ci, :],
                                             start=False,
                                             stop=(ci == ct - 1))
                        # h = relu(acc * dinv_d)
                        hb = agev.tile([P, H], bfl if do_conv2 else odt,
                                       tag="hb")
                        nc.vector.tensor_scalar(
                            out=hb[:], in0=acc[:],
                            scalar1=dinv_sb[:, t:t + 1], scalar2=0.0,
                            op0=mybir.AluOpType.mult,
                            op1=mybir.AluOpType.max)
                        if do_conv2:
                            ht = agev.tile([P, H], bfl, tag="ht")
                            for k in range(2):
                                pt = agp2.tile([P, P], bfl, space="PSUM",
                                               tag="pt")
                                nc.tensor.transpose(
                                    pt[:], hb[:, k * P:(k + 1) * P], ident[:])
                                nc.vector.tensor_copy(
                                    ht[:, k * P:(k + 1) * P], pt[:])
                            pz = agp2.tile([P, H], f32, tag="pz")
                            for k in range(2):
                                nc.tensor.matmul(
                                    pz[:], lhsT=ht[:, k * P:(k + 1) * P],
                                    rhs=wc2_sb[:, k, :],
                                    start=(k == 0), stop=(k == 1))
                            res = agev.tile([P, H], odt, tag="res")
                            nc.vector.tensor_scalar(
                                out=res[:], in0=pz[:],
                                scalar1=dinv_sb[:, t:t + 1], scalar2=None,
                                op0=mybir.AluOpType.mult)
                        else:
                            res = hb
                        nc.sync.dma_start(out_dram[t * P:(t + 1) * P, :],
                                          res[:])
                        if post_tile is not None:
                            post_tile(t)

            lo_off_c = [0]
            for t in range(T):
                lo_off_c.append(lo_off_c[-1] + nlo16[t] // 16)
            hi_off_c = [0]
            for t in range(T):
                hi_off_c.append(hi_off_c[-1] + nhi16[t] // 16)

            half_t = cfg.HSH // P - 1   # last tile of the A half

            def post1(t):
                if t == half_t:
                    nc.gpsimd.collective_compute(
                        "AllGather", mybir.AluOpType.bypass,
                        replica_groups=rg,
                        ins=[z1in[:cfg.HSH, :]], outs=[Z1a[:]])

            agg_layer(Z0a, Z0b, z0in, b1_sb, z1in, do_conv2=True, lname="a",
                      mdt=fp8, odt=fp8, post_tile=post1)
            nc.gpsimd.collective_compute(
                "AllGather", mybir.AluOpType.bypass, replica_groups=rg,
                ins=[z1in[cfg.HSH:, :]], outs=[Z1b[:]])

            agg_layer(Z1a, Z1b, z1in, b2_sb, h2in, do_conv2=False, lname="b",
                      mdt=fp8, odt=bfl, post_tile=None)

            # ---------------- head
            with tc.tile_pool(name="hdsb", bufs=2) as hdsb, \
                 tc.tile_pool(name="hdps", bufs=2, space="PSUM") as hdps:
                zt0 = hdsb.tile([P, BMAX], bfl, tag="zt0")
                zt1 = hdsb.tile([P, BMAX], bfl, tag="zt1")
                for j in range(BCH2):
                    g = hdsb.tile([P, H], bfl, tag="hg")
                    nc.gpsimd.indirect_dma_start(
                        out=g[:], out_offset=None, in_=h2in[:],
                        in_offset=bass.IndirectOffsetOnAxis(
                            ap=vidx_sb[:, j:j + 1], axis=0))
                    for k in range(2):
                        pt = hdps.tile([P, P], bfl, space="PSUM", tag="hpt")
                        nc.tensor.transpose(pt[:], g[:, k * P:(k + 1) * P],
                                            ident[:])
                        dstt = zt0 if k == 0 else zt1
                        nc.vector.tensor_copy(
                            dstt[:, j * P:(j + 1) * P], pt[:])
                for b0 in range(0, BMAX, 512):
                    bw = min(512, BMAX - b0)
                    ph1 = hdps.tile([P, 512], f32, tag="ph1")
                    nc.tensor.matmul(ph1[:, :bw], lhsT=wh1_sb[:, 0, :],
                                     rhs=zt0[:, b0:b0 + bw],
                                     start=True, stop=False)
                    nc.tensor.matmul(ph1[:, :bw], lhsT=wh1_sb[:, 1, :],
                                     rhs=zt1[:, b0:b0 + bw],
                                     start=False, stop=False)
                    nc.tensor.matmul(ph1[:, :bw], lhsT=wh1_sb[:OH, 2, :],
                                     rhs=ohT_sb[:, b0:b0 + bw],
                                     start=False, stop=True)
                    a1 = hdsb.tile([P, 512], bfl, tag="a1")
                    nc.scalar.activation(a1[:, :bw], ph1[:, :bw],
                                         mybir.ActivationFunctionType.Relu,
                                         bias=bh1_sb[:])
                    ph2 = hdps.tile([HH // 2, 512], f32, tag="ph2")
                    nc.tensor.matmul(ph2[:, :bw], lhsT=wh2_sb[:],
                                     rhs=a1[:, :bw], start=True, stop=True)
                    a2 = hdsb.tile([HH // 2, 512], bfl, tag="a2")
                    nc.scalar.activation(a2[:, :bw], ph2[:, :bw],
                                         mybir.ActivationFunctionType.Relu,
                                         bias=bh2_sb[:])
                    ph3 = hdps.tile([1, 512], f32, tag="ph3")
                    nc.tensor.matmul(ph3[:, :bw], lhsT=wh3_sb[:],
                                     rhs=a2[:, :bw], start=True, stop=True)
                    osb = hdsb.tile([1, 512], f32, tag="osb")
                    nc.vector.tensor_scalar_add(osb[:, :bw], ph3[:, :bw],
                                                bh3_sb[:, :1])
                    nc.sync.dma_start(out[:, b0:b0 + bw], osb[:, :bw])

    nc.compile()
    return nc


# ------------------------------------------------------------------ driver

_CACHE = {}


def _get_program(cfg, meta):
    key = (cfg.N, cfg.E, cfg.D_IN, cfg.B, meta["CL"], meta["CH"],
           meta["BMAX"], meta["nlo16"], meta["nhi16"])
    if key not in _CACHE:
        _CACHE[key] = build_program(cfg, meta)
    return _CACHE[key]


def assemble_out(cfg, meta, results):
    full = np.zeros(cfg.B, np.float32)
    for q in range(NCORES):
        pos = meta["positions"][q]
        vals = np.asarray(results[q]["out"]).reshape(meta["BMAX"])
        full[pos] = vals[:len(pos)]
    return full


def kernel(**inputs):
    cfg = REAL
    in_maps, meta = host_prep(cfg, **inputs)
    nc = _get_program(cfg, meta)
    from concourse import bass_utils
    res = bass_utils.run_bass_kernel_spmd(
        nc, in_maps, core_ids=list(range(NCORES)))
    return assemble_out(cfg, meta, res.results)
